# revision 11
# baseline (speedup 1.0000x reference)
"""De-stationarized attention kernel for Trainium2 (8 NeuronCores).

Problem: B=4, L=S=2048, H=8, E=64
    logits = (q @ k^T) * tau * scale + delta * scale      (scale = 1/sqrt(E))
    a      = softmax(logits, axis=-1)                     [B, H, L, S]  (output)
    out    = a @ v                                        [B, L, H, E]  (output)

Sharding: 32 (b, h) pairs over 8 cores -> core i handles b = i//2,
h in [4*(i%2), 4*(i%2)+4). Each core computes its 4 heads fully.

Device-side layout strategy: everything is computed in TRANSPOSED
orientation (s on partitions, l on free dim):
  - lhsT = k1T[65, s]: rows 0-63 = k^T, row 64 = scale*delta[s]
    rhs  = q1T[65, l]: rows 0-63 = (scale*tau*q)^T, row 64 = ones
    => one matmul produces logits^T[s, l] with tau/delta folded in.
  - exp on ACT (no bias needed, so one activation instr spans several
    s-tiles -> amortizes the per-instruction overhead).
  - AV: out^T[d, l] = sum_s v1[s, d] * p^T[s, l] with v1 = [v | ones]:
    row 64 of the psum accumulator is the softmax denominator Z[l].
  - a^T = p^T * (1/Z) broadcast; written to DRAM with l contiguous
    (2KB lines). The host transposes a^T -> a during unsharding.

tau and the 1/sqrt(E) scale are folded into q on the host; delta is
pre-scaled on the host.
"""

import numpy as np

B, L, H, E = 4, 2048, 8, 64
N_CORES = 8
HEADS_PER_CORE = 4
LT = L // 128   # 16 l-tiles
ST = L // 128   # 16 s-tiles
NQ = 4          # l-quarters of 512
QW = L // NQ    # 512

_COMPILED = None

# test-harness knobs (the grading harness never touches these)
TRACE = False
LAST_RESULTS = None


def _split_oversized_waits(nc, mybir, bass_rust, max_waits=1):
    """This walrus build rejects >1 sync wait per instruction; move excess
    waits onto same-engine NOPs inserted immediately before."""
    n_split = 0
    for func in nc.m.functions:
        for block in func.blocks:
            new_insts = []
            dirty = False
            for inst in block.instructions:
                si = inst.sync_info
                waits = list(si.on_wait) if si is not None else []
                if len(waits) > max_waits:
                    dirty = True
                    n_split += 1
                    k = 0
                    i = 0
                    while len(waits) - k > max_waits:
                        chunk = waits[k:k + max_waits]
                        k += max_waits
                        i += 1
                        nop = mybir.InstNoOp(
                            name=f"{inst.name}-wsplit{i}", ins=[], outs=[])
                        nop.engine = inst.engine
                        nop.sync_info = bass_rust.SyncInfo(
                            on_wait=chunk, on_update=[])
                        new_insts.append(nop)
                    inst.sync_info = bass_rust.SyncInfo(
                        on_wait=waits[k:], on_update=list(si.on_update))
                new_insts.append(inst)
            if dirty:
                block.instructions = new_insts
    return n_split


def _build(split_waits=True):
    import concourse.bass as bass
    import concourse.mybir as mybir
    import bass_rust
    from concourse.tile import TileContext
    from concourse.masks import make_identity

    f32 = mybir.dt.float32
    nc = bass.Bass()

    qs = nc.dram_tensor("qs", [L, HEADS_PER_CORE, E], f32, kind="ExternalInput")
    ks = nc.dram_tensor("ks", [L, HEADS_PER_CORE, E], f32, kind="ExternalInput")
    vs = nc.dram_tensor("vs", [L, HEADS_PER_CORE, E], f32, kind="ExternalInput")
    ds = nc.dram_tensor("ds", [L], f32, kind="ExternalInput")
    aT = nc.dram_tensor("aT", [HEADS_PER_CORE, L, L], f32, kind="ExternalOutput")
    outT = nc.dram_tensor("outT", [HEADS_PER_CORE, E, L], f32,
                          kind="ExternalOutput")

    with TileContext(nc) as tc:
        with (
            tc.tile_pool(name="persist", bufs=1) as persist,
            tc.tile_pool(name="qk_T", bufs=2) as qkT_pool,
            tc.tile_pool(name="pT", bufs=16) as pT_pool,
            tc.tile_pool(name="rz", bufs=3) as rz_pool,
            tc.tile_pool(name="ot", bufs=2) as ot_pool,
            tc.tile_pool(name="tp_ps", bufs=1, space="PSUM") as tp_ps,
            tc.tile_pool(name="qk_ps", bufs=2, space="PSUM") as qk_ps,
            tc.tile_pool(name="av_ps", bufs=2, space="PSUM") as av_ps,
        ):
            ident = persist.tile([128, 128], f32)
            make_identity(nc, ident)
            ones_row = persist.tile([1, 128], f32)
            nc.gpsimd.memset(ones_row, 1.0)

            # Stage all inputs (1KB lines, one DMA per tensor).
            q_nat = persist.tile([128, LT, HEADS_PER_CORE, E], f32)
            k_nat = persist.tile([128, LT, HEADS_PER_CORE, E], f32)
            v_nat = persist.tile([128, ST, HEADS_PER_CORE, E], f32)
            v1 = persist.tile([128, ST, HEADS_PER_CORE, E + 1], f32)
            nc.sync.dma_start(
                out=q_nat.rearrange("p t h e -> p t (h e)"),
                in_=qs.rearrange("(t p) h e -> p t (h e)", p=128))
            nc.sync.dma_start(
                out=k_nat.rearrange("p t h e -> p t (h e)"),
                in_=ks.rearrange("(t p) h e -> p t (h e)", p=128))
            nc.sync.dma_start(
                out=v_nat.rearrange("p t h e -> p t (h e)"),
                in_=vs.rearrange("(t p) h e -> p t (h e)", p=128))
            nc.vector.tensor_copy(out=v1[:, :, :, 0:E], in_=v_nat)
            nc.gpsimd.memset(v1[:, :, :, E:E + 1], 1.0)

            for j in range(HEADS_PER_CORE):
                # ---- build k1T [65, 2048] and q1T [65, 2048] ----
                k1T = qkT_pool.tile([E + 1, L], f32, tag="k1T")
                q1T = qkT_pool.tile([E + 1, L], f32, tag="q1T")
                for src, dstT in ((k_nat, k1T), (q_nat, q1T)):
                    for c in range(2):  # chunks of 8 tiles = 1024 cols
                        tp = tp_ps.tile([E, 1024], f32, tag="tp")
                        for t in range(8):
                            nc.tensor.transpose(
                                out=tp[:, t * 128:(t + 1) * 128],
                                in_=src[:, c * 8 + t, j, :],
                                identity=ident,
                            )
                        nc.vector.tensor_copy(
                            out=dstT[0:E, c * 1024:(c + 1) * 1024], in_=tp)
                nc.sync.dma_start(
                    out=k1T[E:E + 1, :],
                    in_=ds.rearrange("(o s) -> o s", o=1))
                nc.gpsimd.memset(q1T[E:E + 1, :], 1.0)

                # ---- main loop: l-quarters x s-tile-pairs ----
                o_stage = ot_pool.tile([E, L], f32, tag="o_stage")
                for q in range(NQ):
                    av = av_ps.tile([E + 1, QW], f32, tag="av")
                    pT_chunks = []

                    def emit_qk(sp):
                        qk = qk_ps.tile([128, 1024], f32, tag="qk")
                        for u in range(2):
                            st = sp * 2 + u
                            nc.tensor.matmul(
                                qk[:, u * 512:(u + 1) * 512],
                                lhsT=k1T[:, st * 128:(st + 1) * 128],
                                rhs=q1T[:, q * QW:(q + 1) * QW],
                                start=True, stop=True,
                            )
                        pT = pT_pool.tile([128, 2, 512], f32, tag="pT")
                        nc.scalar.activation(
                            out=pT.rearrange("p u l -> p (u l)"),
                            in_=qk,
                            func=mybir.ActivationFunctionType.Exp,
                        )
                        pT_chunks.append(pT)

                    def emit_av(sp):
                        pT = pT_chunks[sp]
                        for u in range(2):
                            st = sp * 2 + u
                            nc.tensor.matmul(
                                av,
                                lhsT=v1[:, st, j, :],
                                rhs=pT[:, u, :],
                                start=(st == 0), stop=(st == ST - 1),
                                skip_group_check=True,
                            )

                    # skew by one step: PE computes QK(sp+1) while ACT
                    # runs exp(sp), so PE never stalls on the exp.
                    emit_qk(0)
                    for sp in range(1, ST // 2):
                        emit_qk(sp)
                        emit_av(sp - 1)
                    emit_av(ST // 2 - 1)
                    # av row 64 = Z for this l-quarter; broadcast 1/Z to all
                    # 128 partitions via a K=1 matmul (ones ⊗ rz).
                    rz = rz_pool.tile([1, QW], f32, tag="rz")
                    nc.vector.reciprocal(out=rz, in_=av[E:E + 1, :])
                    rzps = qk_ps.tile([128, 1024], f32, tag="qk")
                    nc.tensor.matmul(
                        rzps[:, 0:QW], lhsT=ones_row, rhs=rz,
                        start=True, stop=True)
                    rzrep = rz_pool.tile([128, QW], f32, tag="rzrep")
                    nc.vector.tensor_copy(out=rzrep, in_=rzps[:, 0:QW])

                    # normalize out^T chunk (psum -> sbuf fused multiply)
                    nc.vector.tensor_mul(
                        out=o_stage[:, q * QW:(q + 1) * QW],
                        in0=av[0:E, :], in1=rzrep[0:E, :])

                    # normalize a^T chunks in place, then store
                    for sp in range(ST // 2):
                        pT = pT_chunks[sp]
                        rz2 = bass.AP(
                            tensor=rzrep.tensor, offset=rzrep.offset,
                            ap=[list(rzrep.ap[0]), [0, 2]] + list(rzrep.ap[1:]))
                        nc.vector.tensor_mul(out=pT, in0=pT, in1=rz2)
                        dst = aT[j, sp * 256:(sp + 1) * 256,
                                 q * QW:(q + 1) * QW]
                        nc.sync.dma_start(
                            out=dst.rearrange("(u p) l -> p u l", u=2),
                            in_=pT)
                nc.sync.dma_start(out=outT[j], in_=o_stage)

    if split_waits:
        _split_oversized_waits(nc, mybir, bass_rust)
    return nc


def _get_compiled():
    global _COMPILED
    if _COMPILED is None:
        _COMPILED = _build()
    return _COMPILED


def kernel(q, k, v, tau, delta):
    from concourse.bass_utils import run_bass_kernel_spmd

    nc = _get_compiled()
    scale = np.float32(1.0 / np.sqrt(np.float32(E)))

    in_maps = []
    for core in range(N_CORES):
        b = core // 2
        h0 = (core % 2) * HEADS_PER_CORE
        qs = np.ascontiguousarray(
            q[b, :, h0:h0 + HEADS_PER_CORE, :]
        ) * (scale * np.float32(tau[b, 0]))
        ks = np.ascontiguousarray(k[b, :, h0:h0 + HEADS_PER_CORE, :])
        vs = np.ascontiguousarray(v[b, :, h0:h0 + HEADS_PER_CORE, :])
        dsv = (delta[b] * scale).astype(np.float32)
        in_maps.append({"qs": qs.astype(np.float32), "ks": ks, "vs": vs,
                        "ds": dsv})

    global LAST_RESULTS
    res = run_bass_kernel_spmd(nc, in_maps, core_ids=list(range(N_CORES)),
                               trace=TRACE)
    LAST_RESULTS = res

    a = np.empty((B, H, L, L), np.float32)
    out = np.empty((B, L, H, E), np.float32)
    for core in range(N_CORES):
        b = core // 2
        h0 = (core % 2) * HEADS_PER_CORE
        aT_res = res.results[core]["aT"]      # [4, s, l]
        oT_res = res.results[core]["outT"]    # [4, d, l]
        for j in range(HEADS_PER_CORE):
            a[b, h0 + j] = aT_res[j].T
            out[:, :, h0 + j, :][b] = oT_res[j].T
    return out, a


# revision 22
# speedup vs baseline: 1.4554x; 1.4554x over previous
"""De-stationarized attention kernel for Trainium2 (8 NeuronCores).

Problem: B=4, L=S=2048, H=8, E=64
    logits = (q @ k^T) * tau * scale + delta * scale      (scale = 1/sqrt(E))
    a      = softmax(logits, axis=-1)                     [B, H, L, S]  (output)
    out    = a @ v                                        [B, L, H, E]  (output)

Sharding: 32 (b, h) pairs over 8 cores -> core i handles b = i//2,
h in [4*(i%2), 4*(i%2)+4). Each core computes its 4 heads fully.

Device-side layout strategy: everything is computed in TRANSPOSED
orientation (s on partitions, l on free dim):
  - lhsT = k1T[65, s]: rows 0-63 = k^T, row 64 = scale*delta[s]
    rhs  = q1T[65, l]: rows 0-63 = (scale*tau*q)^T, row 64 = ones
    => one matmul produces logits^T[s, l] with tau/delta folded in.
    Matmuls run as float32r (1 cycle/row vs 4 for fp32).
  - exp on ACT (no bias needed, so one activation instr spans two
    s-tiles -> amortizes the per-instruction overhead).
  - AV: out^T[d, l] = sum_s v1[s, d] * p^T[s, l] with v1 = [v | ones]:
    row 64 of the psum accumulator is the softmax denominator Z[l].
  - a^T = p^T * (1/Z) broadcast; one l-quarter of p^T lives in a single
    [128, 16, 512] tile so the store is one 4MB DMA with 2KB lines.
    The host transposes a^T -> a during unsharding.

tau and the 1/sqrt(E) scale are folded into q on the host; delta is
pre-scaled on the host.
"""

import numpy as np

B, L, H, E = 4, 2048, 8, 64
N_CORES = 8
HEADS_PER_CORE = 4
LT = L // 128   # 16 l-tiles
ST = L // 128   # 16 s-tiles
NQ = 4          # l-quarters of 512
QW = L // NQ    # 512

_COMPILED = None

# test-harness knobs (the grading harness never touches these)
TRACE = False
LAST_RESULTS = None


def _split_oversized_waits(nc, mybir, bass_rust, max_waits=1):
    """This walrus build rejects >1 sync wait per instruction; move excess
    waits onto same-engine NOPs inserted immediately before."""
    n_split = 0
    for func in nc.m.functions:
        for block in func.blocks:
            new_insts = []
            dirty = False
            for inst in block.instructions:
                si = inst.sync_info
                waits = list(si.on_wait) if si is not None else []
                if len(waits) > max_waits:
                    dirty = True
                    n_split += 1
                    k = 0
                    i = 0
                    while len(waits) - k > max_waits:
                        chunk = waits[k:k + max_waits]
                        k += max_waits
                        i += 1
                        nop = mybir.InstNoOp(
                            name=f"{inst.name}-wsplit{i}", ins=[], outs=[])
                        nop.engine = inst.engine
                        nop.sync_info = bass_rust.SyncInfo(
                            on_wait=chunk, on_update=[])
                        new_insts.append(nop)
                    inst.sync_info = bass_rust.SyncInfo(
                        on_wait=waits[k:], on_update=list(si.on_update))
                new_insts.append(inst)
            if dirty:
                block.instructions = new_insts
    return n_split


def _build(split_waits=True):
    import concourse.bass as bass
    import concourse.mybir as mybir
    import bass_rust
    from concourse.tile import TileContext
    from concourse.masks import make_identity

    f32 = mybir.dt.float32
    f32r = mybir.dt.float32r
    nc = bass.Bass()

    qs = nc.dram_tensor("qs", [L, HEADS_PER_CORE, E], f32, kind="ExternalInput")
    ks = nc.dram_tensor("ks", [L, HEADS_PER_CORE, E], f32, kind="ExternalInput")
    vs = nc.dram_tensor("vs", [L, HEADS_PER_CORE, E], f32, kind="ExternalInput")
    ds = nc.dram_tensor("ds", [L], f32, kind="ExternalInput")
    aT = nc.dram_tensor("aT", [HEADS_PER_CORE, L, L], f32, kind="ExternalOutput")
    outT = nc.dram_tensor("outT", [HEADS_PER_CORE, E, L], f32,
                          kind="ExternalOutput")

    with TileContext(nc) as tc:
        with (
            tc.tile_pool(name="persist", bufs=1) as persist,
            tc.tile_pool(name="qk_T", bufs=2) as qkT_pool,
            tc.tile_pool(name="pT", bufs=2) as pT_pool,
            tc.tile_pool(name="rz", bufs=2) as rz_pool,
            tc.tile_pool(name="ot", bufs=1) as ot_pool,
            tc.tile_pool(name="tp_ps", bufs=1, space="PSUM") as tp_ps,
            tc.tile_pool(name="qk_ps", bufs=2, space="PSUM") as qk_ps,
            tc.tile_pool(name="av_ps", bufs=2, space="PSUM") as av_ps,
        ):
            ident = persist.tile([128, 128], f32)
            make_identity(nc, ident)
            ones_row = persist.tile([1, 128], f32)
            nc.gpsimd.memset(ones_row, 1.0)

            # Stage all inputs (1KB lines for q/k).
            q_nat = persist.tile([128, LT, HEADS_PER_CORE, E], f32)
            k_nat = persist.tile([128, LT, HEADS_PER_CORE, E], f32)
            v_nat = persist.tile([128, ST, HEADS_PER_CORE, E + 1], f32)
            v1 = persist.tile([128, ST, HEADS_PER_CORE, E + 1], f32r)
            # delta staged on partition 64 so the copy into k1T row 64 is
            # lane-aligned (f32 -> f32r cast happens on the copy)
            ds_sb = persist.tile([E + 1, L], f32)
            nc.sync.dma_start(
                out=q_nat.rearrange("p t h e -> p t (h e)"),
                in_=qs.rearrange("(t p) h e -> p t (h e)", p=128))
            nc.sync.dma_start(
                out=k_nat.rearrange("p t h e -> p t (h e)"),
                in_=ks.rearrange("(t p) h e -> p t (h e)", p=128))
            for jj in range(HEADS_PER_CORE):
                nc.sync.dma_start(
                    out=v_nat[:, :, jj, 0:E],
                    in_=vs[:, jj, :].rearrange("(t p) e -> p t e", p=128))
            nc.sync.dma_start(out=ds_sb[E:E + 1, :],
                              in_=ds.rearrange("(o s) -> o s", o=1))
            nc.gpsimd.memset(v_nat[:, :, :, E:E + 1], 1.0)
            nc.vector.tensor_copy(out=v1, in_=v_nat)

            for j in range(HEADS_PER_CORE):
                # ---- build k1T [65, 2048] and q1T [65, 2048] ----
                k1T = qkT_pool.tile([E + 1, L], f32r, tag="k1T")
                q1T = qkT_pool.tile([E + 1, L], f32r, tag="q1T")
                for src, dstT, is_q in ((k_nat, k1T, False), (q_nat, q1T, True)):
                    for c in range(2):  # chunks of 8 tiles = 1024 cols
                        tp = tp_ps.tile([E + 1, 1024], f32, tag="tp")
                        for t in range(8):
                            nc.tensor.transpose(
                                out=tp[0:E, t * 128:(t + 1) * 128],
                                in_=src[:, c * 8 + t, j, :],
                                identity=ident,
                            )
                        if is_q:
                            # ones row folded in via the f32 psum staging
                            nc.vector.memset(tp[E:E + 1, :], 1.0)
                            nc.vector.tensor_copy(
                                out=dstT[:, c * 1024:(c + 1) * 1024], in_=tp)
                        else:
                            nc.vector.tensor_copy(
                                out=dstT[0:E, c * 1024:(c + 1) * 1024],
                                in_=tp[0:E, :])
                nc.vector.tensor_copy(out=k1T[E:E + 1, :],
                                      in_=ds_sb[E:E + 1, :])

                # ---- main loop: l-quarters x s-tile-pairs ----
                o_stage = ot_pool.tile([E, L], f32, tag="o_stage")
                for q in range(NQ):
                    av = av_ps.tile([E + 1, QW], f32, tag="av")
                    pTq = pT_pool.tile([128, ST, QW], f32r, tag="pT")

                    def emit_qk(sp):
                        qk = qk_ps.tile([128, 1024], f32, tag="qk")
                        for u in range(2):
                            st = sp * 2 + u
                            nc.tensor.matmul(
                                qk[:, u * 512:(u + 1) * 512],
                                lhsT=k1T[:, st * 128:(st + 1) * 128],
                                rhs=q1T[:, q * QW:(q + 1) * QW],
                                start=True, stop=True,
                            )
                        nc.scalar.activation(
                            out=pTq[:, sp * 2:(sp + 1) * 2, :]
                            .rearrange("p u l -> p (u l)"),
                            in_=qk,
                            func=mybir.ActivationFunctionType.Exp,
                        )

                    def emit_av(sp):
                        for u in range(2):
                            st = sp * 2 + u
                            nc.tensor.matmul(
                                av,
                                lhsT=v1[:, st, j, :],
                                rhs=pTq[:, st, :],
                                start=(st == 0), stop=(st == ST - 1),
                                skip_group_check=True,
                            )

                    # skew by one step: PE computes QK(sp+1) while ACT
                    # runs exp(sp), so PE never stalls on the exp.
                    emit_qk(0)
                    for sp in range(1, ST // 2):
                        emit_qk(sp)
                        emit_av(sp - 1)
                    emit_av(ST // 2 - 1)

                    # av row 64 = Z for this l-quarter. Build a clean
                    # [128, 1024] tile of 1/Z (duplicated halves) via a
                    # K=1 broadcast matmul, so the normalize multiplies
                    # below use plain contiguous APs.
                    rz = rz_pool.tile([1, 2, QW], f32, tag="rz")
                    for u in range(2):
                        nc.vector.reciprocal(out=rz[:, u, :],
                                             in_=av[E:E + 1, :])
                    rzps = qk_ps.tile([128, 1024], f32, tag="qk")
                    for u in range(2):
                        nc.tensor.matmul(
                            rzps[:, u * QW:(u + 1) * QW], lhsT=ones_row,
                            rhs=rz[:, u, :], start=True, stop=True)
                    rzrep = rz_pool.tile([128, 2, QW], f32, tag="rzrep")
                    nc.vector.tensor_copy(
                        out=rzrep.rearrange("p u l -> p (u l)"), in_=rzps)

                    # normalize out^T chunk (psum -> sbuf fused multiply)
                    nc.vector.tensor_mul(
                        out=o_stage[:, q * QW:(q + 1) * QW],
                        in0=av[0:E, :], in1=rzrep[0:E, 0, :])

                    # normalize a^T in place, then one 4MB store (the
                    # f32r tile is bit-identical to f32, so bitcast views
                    # keep the normalized output at full f32 precision)
                    for sp in range(ST // 2):
                        view = pTq[:, sp * 2:(sp + 1) * 2, :] \
                            .rearrange("p u l -> p (u l)")
                        nc.vector.tensor_mul(
                            out=view, in0=view,
                            in1=rzrep.rearrange("p u l -> p (u l)"))
                    nc.sync.dma_start(
                        out=aT[j, :, q * QW:(q + 1) * QW]
                        .rearrange("(t p) l -> p t l", p=128),
                        in_=pTq.bitcast(f32))
                nc.sync.dma_start(out=outT[j], in_=o_stage)

    if split_waits:
        _split_oversized_waits(nc, mybir, bass_rust)
    return nc


def _get_compiled():
    global _COMPILED
    if _COMPILED is None:
        _COMPILED = _build()
    return _COMPILED


def kernel(q, k, v, tau, delta):
    from concourse.bass_utils import run_bass_kernel_spmd

    nc = _get_compiled()
    scale = np.float32(1.0 / np.sqrt(np.float32(E)))

    in_maps = []
    for core in range(N_CORES):
        b = core // 2
        h0 = (core % 2) * HEADS_PER_CORE
        qs = np.ascontiguousarray(
            q[b, :, h0:h0 + HEADS_PER_CORE, :]
        ) * (scale * np.float32(tau[b, 0]))
        ks = np.ascontiguousarray(k[b, :, h0:h0 + HEADS_PER_CORE, :])
        vs = np.ascontiguousarray(v[b, :, h0:h0 + HEADS_PER_CORE, :])
        dsv = (delta[b] * scale).astype(np.float32)
        in_maps.append({"qs": qs.astype(np.float32), "ks": ks, "vs": vs,
                        "ds": dsv})

    global LAST_RESULTS
    res = run_bass_kernel_spmd(nc, in_maps, core_ids=list(range(N_CORES)),
                               trace=TRACE)
    LAST_RESULTS = res

    a = np.empty((B, H, L, L), np.float32)
    out = np.empty((B, L, H, E), np.float32)
    for core in range(N_CORES):
        b = core // 2
        h0 = (core % 2) * HEADS_PER_CORE
        aT_res = res.results[core]["aT"]      # [4, s, l]
        oT_res = res.results[core]["outT"]    # [4, d, l]
        for j in range(HEADS_PER_CORE):
            a[b, h0 + j] = aT_res[j].T
            out[:, :, h0 + j, :][b] = oT_res[j].T
    return out, a
